# revision 21
# baseline (speedup 1.0000x reference)
"""Trainium2 Bass kernel for nn_CausalMultiresConv1d.

Everything before the final GELU is linear: the whole multires stack is
one combined causal FIR filter per channel, F[c, 0:766], computed on the
host as the impulse response of the reference's linear part.

    out[b, c, n] = gelu( sum_tau F[c, tau] * x[b, c, n - tau] )

Sharding: pure data parallel - 1 batch element per NeuronCore (B=8).

Per-core algorithm (transposed layout so the conv is a PE matmul):
  x[c, 16384*h + 128*t + p]  ->  xt[p, 128*(MH + t) + 64*h + c]   (host)
  i.e. positions-within-block on partitions, (block, half, channel) on
  columns, with MH leading halo blocks per half (zeros for half 0, the
  tail of half 0 for half 1) so the causal history is always in-slice.

  For each channel, the FIR becomes M_c banded matmuls accumulated in
  PSUM:   Y_c[p, (t,h)] = sum_m A_m^c.T @ xt[:, block t-m, (h,c)]
  with A_m^c[q, p] = F[c, p - q + 128 m]  (128x128 Toeplitz bands, bf16).
  M_c is per-channel: bands whose tail energy is negligible are dropped
  (total truncation error ~2e-3 relative, tolerance is 2e-2).

  ACT drains PSUM with exact GELU into a transposed bf16 buffer; PE
  transposes each 128-column block back to natural [64h+c, col] layout;
  ACT/DVE/Pool drain those to fp32 and the result DMAs out.
"""

import numpy as np
import ml_dtypes

import concourse.bass as bass
import concourse.mybir as mybir
from concourse.bass_utils import run_bass_kernel_spmd
from concourse.tile import TileContext

# The walrus build here rejects instructions carrying more than one sync-wait
# ("Too many sync wait commands"). Tile's kernel-tail drain attaches a wait for
# every outstanding semaphore to a single SP Drain. _TC splits them: hoist all
# but the last wait onto dedicated single-wait NOPs preceding the drain.


class _TC(TileContext):
    def __exit__(self, *a):
        r = super().__exit__(*a)
        _split_multi_waits(self.nc)
        return r


def _split_multi_waits(nc):
    n = 0
    for fn in nc.m.functions:
        for blk in fn.blocks:
            insts = getattr(blk, "instructions", None)
            if insts is None:
                continue
            new = []
            for inst in insts:
                si = getattr(inst, "sync_info", None)
                waits = list(si.on_wait) if si is not None and si.on_wait else []
                if len(waits) > 1:
                    for j, wcmd in enumerate(waits[:-1]):
                        nop = mybir.InstNoOp(
                            name=f"{inst.name}-hw{j}", engine=inst.engine
                        )
                        nop.sync_info = mybir.SyncInfo(
                            on_wait=[wcmd], on_update=[]
                        )
                        new.append(nop)
                        n += 1
                    inst.sync_info = mybir.SyncInfo(
                        on_wait=[waits[-1]], on_update=list(si.on_update)
                    )
                new.append(inst)
            blk.instructions[:] = new
    return n


B, C, L = 8, 64, 32768
K, DEPTH = 4, 8
NCORES = 8
NH = 2                  # L-halves packed side by side in the channel dim
HL = L // NH            # 16384 positions per half
NB = HL // 128          # 128 blocks of 128 positions per half
P = 128
FLEN = 766              # combined filter support
MAXM = 7                # max 128-tap bands (covers 766 taps)
TRUNC_THR = 1e-6        # per-channel tail energy cutoff (frac of total)
TSEG = 8                # output blocks per PSUM segment (one 2KB bank)

F32 = mybir.dt.float32
BF16 = mybir.dt.bfloat16


def _combined_filter(h0, h1, w):
    """Impulse response [C, FLEN] of the linear part, in float64."""
    h0d = h0[:, 0, :].astype(np.float64)
    h1d = h1[:, 0, :].astype(np.float64)
    wd = w.astype(np.float64)

    def dconv(r, h, d):
        out = np.zeros_like(r)
        for k in range(K):
            s = (K - 1 - k) * d
            out[:, s:] += h[:, k:k + 1] * r[:, :FLEN - s]
        return out

    r = np.zeros((C, FLEN))
    r[:, 0] = 1.0
    y = np.zeros((C, FLEN))
    d = 1
    for i in range(DEPTH, 0, -1):
        y += wd[:, i][:, None] * dconv(r, h1d, d)
        r = dconv(r, h0d, d)
        d *= 2
    y += wd[:, 0][:, None] * r
    y[:, 0] += wd[:, -1]
    return y


def _choose_mc(F):
    """Per-channel band count. With nb bands, the worst-covered output
    position (po=0 in a block) only sees taps <= 128*(nb-1), so pick the
    smallest nb whose worst-case dropped tail is negligible."""
    E = F * F
    tot = E.sum()
    mc = []
    for c in range(C):
        nb = MAXM
        for M in range(1, MAXM):
            if E[c, 128 * M:].sum() <= TRUNC_THR * tot:
                nb = M + 1
                break
        mc.append(nb)
    return tuple(mc)


def _build_nc(mc, reps=1):
    nc = bass.Bass()
    mh = max(mc) - 1                      # halo blocks
    cw = 2 * mh + 2 * NB                  # columns per channel (halo + data)
    xt_cols = C * cw
    na = sum(mc) + 1                      # band matrices + identity
    xt_in = nc.dram_tensor("xt", [P, xt_cols], BF16, kind="ExternalInput")
    am_in = nc.dram_tensor("am", [P, na * 128], BF16, kind="ExternalInput")
    y_out = nc.dram_tensor("y", [P, HL], BF16, kind="ExternalOutput")

    GELU = mybir.ActivationFunctionType.Gelu

    with _TC(nc) as tc:
        with (
            tc.tile_pool(name="main", bufs=1) as pool,
            tc.tile_pool(name="psum", bufs=1, space="PSUM") as psum_pool,
        ):
            xts = pool.tile([P, xt_cols], BF16, tag="xts")
            ams = pool.tile([P, na * 128], BF16, tag="ams")
            tty = pool.tile([P, NB * 128], BF16, tag="tty")
            ynat = pool.tile([P, NB * 128], BF16, tag="ynat")

            nc.sync.dma_start(out=xts[:], in_=xt_in[:])
            nc.sync.dma_start(out=ams[:], in_=am_in[:])

            ident = ams[:, (na - 1) * 128: na * 128]

            def emit_body():
                # conv: per channel, mc[c] banded matmuls accumulated in
                # PSUM. xt is channel-major with (t, h)-interleaved columns
                # and a private halo per channel, so every rhs is one
                # CONTIGUOUS 256-column window (strided rhs runs ~3x slower
                # on the PE's SBUF read path).
                off = 0
                for c in range(C):
                    ps = psum_pool.tile([P, 128, 2], F32, tag="ps", bufs=4)
                    for m in range(mc[c]):
                        s0 = c * cw + 2 * (mh - m)
                        nc.tensor.matmul(
                            ps[:],
                            lhsT=ams[:, (off + m) * 128: (off + m + 1) * 128],
                            rhs=xts[:, s0: s0 + 2 * NB],
                            start=(m == 0),
                            stop=(m == mc[c] - 1),
                        )
                    off += mc[c]
                    # exact GELU while draining PSUM; tty is CHANNEL-major
                    # (columns c*256 + 2t + h) so this write is contiguous -
                    # strided engine writes run ~4x slower
                    nc.scalar.activation(
                        out=tty[:, c * 256: (c + 1) * 256],
                        in_=ps.rearrange("p a b -> p (a b)"),
                        func=GELU,
                    )

                # transpose each 128-col tile (= one channel x 64 blocks x 2
                # halves); output rows become the (2t+h) interleave that the
                # host unpack untangles. Drain each PSUM segment on DVE
                # (bitcast bf16 pairs to fp32: copies are dtype-agnostic,
                # so this halves the DVE cycles)
                for s in range(NB // TSEG):
                    psb = psum_pool.tile([P, TSEG * 128], BF16, tag="psb",
                                         bufs=4)
                    for i in range(TSEG):
                        t = s * TSEG + i
                        nc.tensor.transpose(
                            psb[:, i * 128: (i + 1) * 128],
                            tty[:, t * 128: (t + 1) * 128],
                            ident,
                        )
                    dst = ynat[:, s * TSEG * 128: (s + 1) * TSEG * 128]
                    nc.vector.tensor_copy(dst.bitcast(F32), psb[:].bitcast(F32))

            if reps == 1:
                emit_body()
            else:
                # in-NEFF rep loop for delta timing: constant instruction
                # count, so huge rep counts stay cheap to compile
                with tc.For_i(0, reps):
                    emit_body()
            nc.sync.dma_start(out=y_out[:], in_=ynat[:])
    return nc


_NC_CACHE = {}


def _get_nc(mc, reps=1):
    key = (mc, reps)
    if key not in _NC_CACHE:
        _NC_CACHE[key] = _build_nc(mc, reps)
    return _NC_CACHE[key]


def _band_matrices(F, mc):
    """[P, (sum(mc)+1)*128] bf16: per-channel Toeplitz bands + identity."""
    na = sum(mc) + 1
    am = np.zeros((P, na * 128), np.float32)
    q = np.arange(128)
    off = 0
    for c in range(C):
        Fz = np.zeros(127 + 128 * MAXM + 128)
        Fz[127: 127 + FLEN] = F[c]
        win = np.lib.stride_tricks.sliding_window_view(Fz, 128)
        for m in range(mc[c]):
            # A[q, p] = F[c, p - q + 128 m]
            am[:, (off + m) * 128: (off + m + 1) * 128] = win[127 + 128 * m - q]
        off += mc[c]
    am[:, (na - 1) * 128: na * 128] = np.eye(128, dtype=np.float32)
    return am.astype(ml_dtypes.bfloat16)


def pack_inputs(x, h0, h1, w):
    F = _combined_filter(h0, h1, w)
    mc = _choose_mc(F)
    mh = max(mc) - 1
    am = _band_matrices(F, mc)

    in_maps = []
    for bi in range(NCORES):
        xr = np.ascontiguousarray(x[bi]).reshape(C, NH, NB, 128)
        # channel-major, (t, h)-interleaved columns with per-channel halo:
        # xt[p, c*cw + 2*(mh + t) + h] = x[bi, c, 16384*h + 128*t + p]
        full = np.zeros((P, C, mh + NB, NH), np.float32)
        full[:, :, mh:, :] = xr.transpose(3, 0, 2, 1)
        # half 1's causal history is half 0's last mh blocks
        full[:, :, :mh, 1] = xr[:, 0, NB - mh:, :].transpose(2, 0, 1)
        xt = full.reshape(P, C * (mh + NB) * NH).astype(ml_dtypes.bfloat16)
        in_maps.append({"xt": xt, "am": am})
    return in_maps, mc


def unpack_outputs(results):
    # device output rows are the (2*t + h) interleave of each transposed
    # channel-major tile; columns are (c, tile-half g, p)
    out = np.empty((B, C, L), np.float32)
    for bi, r in enumerate(results):
        yv = np.asarray(r["y"]).astype(np.float32)
        v = yv.reshape(64, NH, C, 2, 128)            # [tm, h, c, g, p]
        out[bi] = v.transpose(2, 1, 3, 0, 4).reshape(C, L)
    return out


def kernel(x, h0, h1, w, _trace=False):
    import os
    os.environ.setdefault("BASS_NEVER_TRACE", "1")

    x = np.asarray(x, np.float32)
    h0 = np.asarray(h0, np.float32)
    h1 = np.asarray(h1, np.float32)
    w = np.asarray(w, np.float32)

    in_maps, mc = pack_inputs(x, h0, h1, w)
    nc = _get_nc(mc, 1)
    try:
        res = run_bass_kernel_spmd(
            nc, in_maps, core_ids=list(range(NCORES)), trace=_trace,
        )
    except Exception:
        # transient "device unrecoverable" failures have been observed on
        # this fleet; one retry usually succeeds
        res = run_bass_kernel_spmd(
            nc, in_maps, core_ids=list(range(NCORES)), trace=_trace,
        )
    out = unpack_outputs(res.results)
    if _trace:
        return out, res
    return out


# revision 22
# speedup vs baseline: 1.8191x; 1.8191x over previous
"""Trainium2 Bass kernel for nn_CausalMultiresConv1d.

Everything before the final GELU is linear: the whole multires stack is
one combined causal FIR filter per channel, F[c, 0:766], computed on the
host as the impulse response of the reference's linear part.

    out[b, c, n] = gelu( sum_tau F[c, tau] * x[b, c, n - tau] )

Sharding: pure data parallel - 1 batch element per NeuronCore (B=8).

Per-core algorithm (transposed layout so the conv is a PE matmul):
  x[c, 16384*h + 128*t + p]  ->  xt[p, 128*(MH + t) + 64*h + c]   (host)
  i.e. positions-within-block on partitions, (block, half, channel) on
  columns, with MH leading halo blocks per half (zeros for half 0, the
  tail of half 0 for half 1) so the causal history is always in-slice.

  For each channel, the FIR becomes M_c banded matmuls accumulated in
  PSUM:   Y_c[p, (t,h)] = sum_m A_m^c.T @ xt[:, block t-m, (h,c)]
  with A_m^c[q, p] = F[c, p - q + 128 m]  (128x128 Toeplitz bands, bf16).
  M_c is per-channel: bands whose tail energy is negligible are dropped
  (total truncation error ~2e-3 relative, tolerance is 2e-2).

  ACT drains PSUM with exact GELU into a transposed bf16 buffer; PE
  transposes each 128-column block back to natural [64h+c, col] layout;
  ACT/DVE/Pool drain those to fp32 and the result DMAs out.
"""

import numpy as np
import ml_dtypes

import concourse.bass as bass
import concourse.mybir as mybir
from concourse.bass_utils import run_bass_kernel_spmd
from concourse.tile import TileContext

# The walrus build here rejects instructions carrying more than one sync-wait
# ("Too many sync wait commands"). Tile's kernel-tail drain attaches a wait for
# every outstanding semaphore to a single SP Drain. _TC splits them: hoist all
# but the last wait onto dedicated single-wait NOPs preceding the drain.


class _TC(TileContext):
    def __exit__(self, *a):
        r = super().__exit__(*a)
        _split_multi_waits(self.nc)
        return r


def _split_multi_waits(nc):
    n = 0
    for fn in nc.m.functions:
        for blk in fn.blocks:
            insts = getattr(blk, "instructions", None)
            if insts is None:
                continue
            new = []
            for inst in insts:
                si = getattr(inst, "sync_info", None)
                waits = list(si.on_wait) if si is not None and si.on_wait else []
                if len(waits) > 1:
                    for j, wcmd in enumerate(waits[:-1]):
                        nop = mybir.InstNoOp(
                            name=f"{inst.name}-hw{j}", engine=inst.engine
                        )
                        nop.sync_info = mybir.SyncInfo(
                            on_wait=[wcmd], on_update=[]
                        )
                        new.append(nop)
                        n += 1
                    inst.sync_info = mybir.SyncInfo(
                        on_wait=[waits[-1]], on_update=list(si.on_update)
                    )
                new.append(inst)
            blk.instructions[:] = new
    return n


B, C, L = 8, 64, 32768
K, DEPTH = 4, 8
NCORES = 8
NH = 2                  # L-halves packed side by side in the channel dim
HL = L // NH            # 16384 positions per half
NB = HL // 128          # 128 blocks of 128 positions per half
P = 128
FLEN = 766              # combined filter support
MAXM = 7                # max 128-tap bands (covers 766 taps)
TRUNC_THR = 1e-6        # per-channel tail energy cutoff (frac of total)
TSEG = 8                # output blocks per PSUM segment (one 2KB bank)

F32 = mybir.dt.float32
BF16 = mybir.dt.bfloat16


def _combined_filter(h0, h1, w):
    """Impulse response [C, FLEN] of the linear part, in float64."""
    h0d = h0[:, 0, :].astype(np.float64)
    h1d = h1[:, 0, :].astype(np.float64)
    wd = w.astype(np.float64)

    def dconv(r, h, d):
        out = np.zeros_like(r)
        for k in range(K):
            s = (K - 1 - k) * d
            out[:, s:] += h[:, k:k + 1] * r[:, :FLEN - s]
        return out

    r = np.zeros((C, FLEN))
    r[:, 0] = 1.0
    y = np.zeros((C, FLEN))
    d = 1
    for i in range(DEPTH, 0, -1):
        y += wd[:, i][:, None] * dconv(r, h1d, d)
        r = dconv(r, h0d, d)
        d *= 2
    y += wd[:, 0][:, None] * r
    y[:, 0] += wd[:, -1]
    return y


def _choose_mc(F):
    """Per-channel band count. With nb bands, the worst-covered output
    position (po=0 in a block) only sees taps <= 128*(nb-1), so pick the
    smallest nb whose worst-case dropped tail is negligible."""
    E = F * F
    tot = E.sum()
    mc = []
    for c in range(C):
        nb = MAXM
        for M in range(1, MAXM):
            if E[c, 128 * M:].sum() <= TRUNC_THR * tot:
                nb = M + 1
                break
        mc.append(nb)
    return tuple(mc)


def _build_nc(mc, reps=1):
    nc = bass.Bass()
    mh = max(mc) - 1                      # halo blocks
    cw = 2 * mh + 2 * NB                  # columns per channel (halo + data)
    xt_cols = C * cw
    na = sum(mc) + 1                      # band matrices + identity
    xt_in = nc.dram_tensor("xt", [P, xt_cols], BF16, kind="ExternalInput")
    am_in = nc.dram_tensor("am", [P, na * 128], BF16, kind="ExternalInput")
    y_out = nc.dram_tensor("y", [P, HL], BF16, kind="ExternalOutput")

    GELU = mybir.ActivationFunctionType.Gelu

    with _TC(nc) as tc:
        with (
            tc.tile_pool(name="main", bufs=1) as pool,
            tc.tile_pool(name="psum", bufs=1, space="PSUM") as psum_pool,
        ):
            xts = pool.tile([P, xt_cols], BF16, tag="xts")
            ams = pool.tile([P, na * 128], BF16, tag="ams")
            tty = pool.tile([P, NB * 128], BF16, tag="tty")
            ynat = pool.tile([P, NB * 128], BF16, tag="ynat")

            nc.sync.dma_start(out=xts[:], in_=xt_in[:])
            nc.sync.dma_start(out=ams[:], in_=am_in[:])

            ident = ams[:, (na - 1) * 128: na * 128]

            def emit_body():
                # conv: per channel, mc[c] banded matmuls accumulated in
                # PSUM. xt is channel-major with (t, h)-interleaved columns
                # and a private halo per channel, so every rhs is one
                # CONTIGUOUS 256-column window (strided rhs runs ~3x slower
                # on the PE's SBUF read path).
                off = 0
                for c in range(C):
                    ps = psum_pool.tile([P, 128, 2], F32, tag="ps", bufs=4)
                    for m in range(mc[c]):
                        s0 = c * cw + 2 * (mh - m)
                        nc.tensor.matmul(
                            ps[:],
                            lhsT=ams[:, (off + m) * 128: (off + m + 1) * 128],
                            rhs=xts[:, s0: s0 + 2 * NB],
                            start=(m == 0),
                            stop=(m == mc[c] - 1),
                        )
                    off += mc[c]
                    # exact GELU while draining PSUM; tty is CHANNEL-major
                    # (columns c*256 + 2t + h) so this write is contiguous -
                    # strided engine writes run ~4x slower
                    nc.scalar.activation(
                        out=tty[:, c * 256: (c + 1) * 256],
                        in_=ps.rearrange("p a b -> p (a b)"),
                        func=GELU,
                    )

                    # this channel's back-transpose only needs its own gelu
                    # output, so it pipelines right behind the conv instead
                    # of forming a serial tail. Output rows become the
                    # (2t+h) interleave that the host unpack untangles; the
                    # DVE drain moves bf16 pairs bitcast as fp32.
                    psb = psum_pool.tile([P, 256], BF16, tag="psb", bufs=4)
                    for g in range(2):
                        nc.tensor.transpose(
                            psb[:, g * 128: (g + 1) * 128],
                            tty[:, c * 256 + g * 128: c * 256 + (g + 1) * 128],
                            ident,
                        )
                    dst = ynat[:, c * 256: (c + 1) * 256]
                    nc.vector.tensor_copy(dst.bitcast(F32), psb[:].bitcast(F32))

            if reps == 1:
                emit_body()
            else:
                # in-NEFF rep loop for delta timing: constant instruction
                # count, so huge rep counts stay cheap to compile
                with tc.For_i(0, reps):
                    emit_body()
            nc.sync.dma_start(out=y_out[:], in_=ynat[:])
    return nc


_NC_CACHE = {}


def _get_nc(mc, reps=1):
    key = (mc, reps)
    if key not in _NC_CACHE:
        _NC_CACHE[key] = _build_nc(mc, reps)
    return _NC_CACHE[key]


def _band_matrices(F, mc):
    """[P, (sum(mc)+1)*128] bf16: per-channel Toeplitz bands + identity."""
    na = sum(mc) + 1
    am = np.zeros((P, na * 128), np.float32)
    q = np.arange(128)
    off = 0
    for c in range(C):
        Fz = np.zeros(127 + 128 * MAXM + 128)
        Fz[127: 127 + FLEN] = F[c]
        win = np.lib.stride_tricks.sliding_window_view(Fz, 128)
        for m in range(mc[c]):
            # A[q, p] = F[c, p - q + 128 m]
            am[:, (off + m) * 128: (off + m + 1) * 128] = win[127 + 128 * m - q]
        off += mc[c]
    am[:, (na - 1) * 128: na * 128] = np.eye(128, dtype=np.float32)
    return am.astype(ml_dtypes.bfloat16)


def pack_inputs(x, h0, h1, w):
    F = _combined_filter(h0, h1, w)
    mc = _choose_mc(F)
    mh = max(mc) - 1
    am = _band_matrices(F, mc)

    in_maps = []
    for bi in range(NCORES):
        xr = np.ascontiguousarray(x[bi]).reshape(C, NH, NB, 128)
        # channel-major, (t, h)-interleaved columns with per-channel halo:
        # xt[p, c*cw + 2*(mh + t) + h] = x[bi, c, 16384*h + 128*t + p]
        full = np.zeros((P, C, mh + NB, NH), np.float32)
        full[:, :, mh:, :] = xr.transpose(3, 0, 2, 1)
        # half 1's causal history is half 0's last mh blocks
        full[:, :, :mh, 1] = xr[:, 0, NB - mh:, :].transpose(2, 0, 1)
        xt = full.reshape(P, C * (mh + NB) * NH).astype(ml_dtypes.bfloat16)
        in_maps.append({"xt": xt, "am": am})
    return in_maps, mc


def unpack_outputs(results):
    # device output rows are the (2*t + h) interleave of each transposed
    # channel-major tile; columns are (c, tile-half g, p)
    out = np.empty((B, C, L), np.float32)
    for bi, r in enumerate(results):
        yv = np.asarray(r["y"]).astype(np.float32)
        v = yv.reshape(64, NH, C, 2, 128)            # [tm, h, c, g, p]
        out[bi] = v.transpose(2, 1, 3, 0, 4).reshape(C, L)
    return out


def kernel(x, h0, h1, w, _trace=False):
    import os
    os.environ.setdefault("BASS_NEVER_TRACE", "1")

    x = np.asarray(x, np.float32)
    h0 = np.asarray(h0, np.float32)
    h1 = np.asarray(h1, np.float32)
    w = np.asarray(w, np.float32)

    in_maps, mc = pack_inputs(x, h0, h1, w)
    nc = _get_nc(mc, 1)
    try:
        res = run_bass_kernel_spmd(
            nc, in_maps, core_ids=list(range(NCORES)), trace=_trace,
        )
    except Exception:
        # transient "device unrecoverable" failures have been observed on
        # this fleet; one retry usually succeeds
        res = run_bass_kernel_spmd(
            nc, in_maps, core_ids=list(range(NCORES)), trace=_trace,
        )
    out = unpack_outputs(res.results)
    if _trace:
        return out, res
    return out
